# revision 26
# baseline (speedup 1.0000x reference)
"""Deformable-conv Trainium2 kernel (nn_DeformConv_11553462026367).

Strategy: data-parallel over batch — one sample per NeuronCore (8 cores).
Per core:
  1. offsets = 3x3 conv(x, w_offset) on PE (shifted matmuls over padded x)
  2. transpose offsets to position-major layout [p=w, s=h]; compute bilinear
     corner indices + 4 quadrant weights on DVE (mod/clamp arithmetic)
  3. dma_gather: one 1KB descriptor per (k, position) fetches the full
     2x2xC corner patch from a host-prepared pair-interleaved table L
     (fully zero-padded border rows make OOB masking automatic)
  4. quadrant weights are expanded along the channel dim via small PE
     matmuls (w^T x identity-kron-ones), blended on DVE, transposed on
     PE, and reduced against the deform weights via accumulating matmuls.

kernel(**inputs) takes the FULL batch and returns the FULL output.
"""
import sys
sys.path.insert(0, "/opt/trn_rl_repo")

import numpy as np
from contextlib import ExitStack

from concourse import bass, tile
import concourse.bacc as bacc
from concourse.tile import add_dep_helper
import concourse.bass_utils as bass_utils
import concourse.mybir as mybir
from concourse.masks import make_identity

F32 = mybir.dt.float32
F32R = mybir.dt.float32r
BF16 = mybir.dt.bfloat16
I32 = mybir.dt.int32
I16 = mybir.dt.int16
ALU = mybir.AluOpType

# ---- problem constants (hardcoded; kernel.py must be self-contained) ----
B, C, H, W = 8, 64, 128, 128
KK = 9
HW = H * W                 # 16384 positions
LR, LC = 132, 133          # padded gather grid: row=y0+2 in [0,131], col=x0+2 in [0,132]
LROWS = LR * LC + 8        # pad so the final 2-row descriptor stays in bounds
CAST_RNE = True            # HW f32->i32 tensor_copy rounds-to-nearest (sim truncates)
NCORES = 8

# dtype knobs
L_DT = F32                 # gather table dtype
MM_DT = F32R               # PE matmul dtype (float32 or float32r)
TR_DT = F32R               # PE transpose dtype

SG = 16                    # gather chunks per k
SP = HW // 128 // SG       # j-slots per chunk (8) -> 1024 positions per call
NI = 128 * SP              # num_idxs per gather call
KPAIRS = [(0, 1), (2, 3), (4, 5), (6, 7), (8, None)]


def _mm(ap, dt=None):
    dt = dt or MM_DT
    return ap.bitcast(dt) if dt != F32 else ap


def build_kernel(tc, outs, ins, ckpt=None):
    nc = tc.nc
    ctx = ExitStack()
    with ctx:
        # ---------------- constants / persistent tiles ----------------
        const_pool = ctx.enter_context(tc.tile_pool(name="const", bufs=1))
        ident = const_pool.tile([128, 128], F32)
        make_identity(nc, ident[:])

        piota_i = const_pool.tile([128, 1], I32)
        nc.gpsimd.iota(piota_i[:], pattern=[[0, 1]], base=0, channel_multiplier=1)
        piota = const_pool.tile([128, 1], F32)
        nc.vector.tensor_copy(piota[:], piota_i[:])
        siota_i = const_pool.tile([128, 128], I32)
        nc.gpsimd.iota(siota_i[:], pattern=[[1, 128]], base=0, channel_multiplier=0)
        siota = const_pool.tile([128, 128], F32)
        nc.vector.tensor_copy(siota[:], siota_i[:])

        id8_f = const_pool.tile([SP, SP * 64], F32)
        nc.sync.dma_start(id8_f[:], ins["id8ones"])
        id8ones = const_pool.tile([SP, SP * 64], F32R)
        nc.scalar.copy(id8ones[:], id8_f[:])

        wdt_f = const_pool.tile([128, 5 * 64], F32)
        nc.sync.dma_start(
            wdt_f[:].rearrange("p (i m) -> p i m", i=5),
            ins["wdt"].transpose([1, 0, 2]))
        wdt_sb = const_pool.tile([128, 5 * 64], F32R)
        nc.scalar.copy(wdt_sb[:], wdt_f[:])

        woff_f = const_pool.tile([64, KK * 18], F32)
        nc.sync.dma_start(
            woff_f[:].rearrange("p (k o) -> p k o", k=KK),
            ins["woff"].transpose([1, 0, 2]))
        woff_sb = const_pool.tile([64, KK * 18], F32R)
        nc.scalar.copy(woff_sb[:], woff_f[:])

        T_pool = ctx.enter_context(tc.tile_pool(name="persist", bufs=1))
        Ttile = T_pool.tile([128, 128 * 18], F32)          # [p=w, s=h, ch]
        W4 = T_pool.tile([128, KK * SG * 4 * SP], F32)     # [p, k, g, q, s]
        W16all = T_pool.tile([128, KK * SG * 64], I16)     # wrapped idx [p,(k,g,t)]

        msel_f = const_pool.tile([128, 8 * 128], F32)
        nc.sync.dma_start(
            msel_f[:].rearrange("p (a b) -> p a b", a=8),
            ins["msel"].transpose([1, 0, 2]))

        # ---------------- phase A: offsets conv + transpose ----------------
        # All SBUF pools stay open for the whole kernel so tile slots are
        # never reused across DMA-written regions (keeps per-instruction
        # semaphore-wait counts within the ISA limit).
        xpad_pool = ctx.enter_context(tc.tile_pool(name="xpad", bufs=1))
        xr_pool = ctx.enter_context(tc.tile_pool(name="xr", bufs=2))
        offsb_pool = ctx.enter_context(tc.tile_pool(name="offsb", bufs=1))
        gout_pool = ctx.enter_context(tc.tile_pool(name="gout", bufs=2))
        wts_pool = ctx.enter_context(tc.tile_pool(name="wts", bufs=3))
        b2_pool = ctx.enter_context(tc.tile_pool(name="b2", bufs=5))
        btmp_pool = ctx.enter_context(tc.tile_pool(name="btmp", bufs=1))
        rhs_pool = ctx.enter_context(tc.tile_pool(name="rhs", bufs=2))
        osb_pool = ctx.enter_context(tc.tile_pool(name="osb", bufs=2))
        ixp = ctx.enter_context(tc.tile_pool(name="ixtmp", bufs=1))
        touch_pool = ctx.enter_context(tc.tile_pool(name="touch", bufs=2))

        T3 = Ttile[:].rearrange("p (s c) -> p s c", c=18)
        offs_sb = offsb_pool.tile([18, HW // 4], F32)
        xpad = xpad_pool.tile([64, 130 * 130], F32)
        nc.sync.dma_start(xpad[:], ins["xpad"])
        with tc.tile_pool(name="offps", bufs=2, space="PSUM") as offps_pool, \
             tc.tile_pool(name="trps", bufs=2, space="PSUM") as trps_pool:
            woff_v = woff_sb[:].rearrange("p (k o) -> p k o", k=KK)
            for quarter in range(4):
                for hch in range(8):       # chunks of 4 image rows (512 cols)
                    cch = quarter * 8 + hch
                    y0 = cch * 4
                    xr = xr_pool.tile([64, 6 * 130], F32R, tag="xr")
                    nc.scalar.copy(xr[:], xpad[:, y0 * 130:(y0 + 6) * 130])
                    ps = offps_pool.tile([18, 512], F32, tag="offps")
                    for k in range(KK):
                        ky, kx = k // 3, k % 3
                        src = bass.AP(
                            xr.tensor, xr[:].offset + ky * 130 + kx,
                            [[6 * 130, 64], [130, 4], [1, 128]])
                        nc.tensor.matmul(
                            ps[:], woff_v[:, k, :], src,
                            start=(k == 0), stop=(k == KK - 1))
                    nc.scalar.copy(offs_sb[:, hch * 512:(hch + 1) * 512], ps[:])

                for sh4 in range(8):
                    tp = trps_pool.tile([128, 4 * 18], F32, tag="trps")
                    for j4 in range(4):
                        sh = sh4 * 4 + j4
                        nc.tensor.transpose(
                            tp[:, j4 * 18:(j4 + 1) * 18],
                            offs_sb[:, sh * 128:(sh + 1) * 128], ident[:18, :18])
                    s = quarter * 32 + sh4 * 4
                    nc.scalar.copy(
                        T3[:, s:s + 4, :],
                        tp[:].rearrange("p (a c) -> p a c", a=4))

        # ---------------- phase B: index + weight math ----------------
        W4v = W4[:].rearrange("p (k g q s) -> p k g q s", k=KK, g=SG, q=4)
        W16v = W16all[:].rearrange("p (k g t) -> p k g t", k=KK, g=SG)
        msel_v = msel_f[:].rearrange("p (a b) -> p a b", a=8)
        with tc.tile_pool(name="wrps", bufs=2, space="PSUM") as wrp_pool:
            for k in range(KK):
                ky, kx = k // 3, k % 3
                dy = T3[:, :, 2 * k]
                dx = T3[:, :, 2 * k + 1]
                t1 = ixp.tile([128, 128], F32, tag="t1")
                nc.vector.tensor_tensor(t1[:], dy, siota[:], ALU.add)
                ysp8 = ixp.tile([128, 128], F32, tag="ysp8")
                nc.vector.tensor_scalar(ysp8[:], t1[:], float(ky + 7) - (0.5 if CAST_RNE else 0.0), None, ALU.add)
                yint = ixp.tile([128, 128], I32, tag="yint")
                nc.vector.tensor_copy(yint[:], ysp8[:])
                y0f = ixp.tile([128, 128], F32, tag="y0f")
                nc.vector.tensor_copy(y0f[:], yint[:])
                if CAST_RNE:
                    nc.vector.tensor_scalar(ysp8[:], ysp8[:], 0.5, None, ALU.add)
                fy = ixp.tile([128, 128], F32, tag="fy")
                nc.vector.tensor_tensor(fy[:], ysp8[:], y0f[:], ALU.subtract)
                yi = ixp.tile([128, 128], F32, tag="yi")
                nc.vector.tensor_scalar(yi[:], y0f[:], 6.0, 137.0, ALU.max, ALU.min)

                xsp8 = ixp.tile([128, 128], F32, tag="xsp8")
                nc.vector.tensor_scalar(xsp8[:], dx, piota[:], float(kx + 7) - (0.5 if CAST_RNE else 0.0),
                                        ALU.add, ALU.add)
                xint = ixp.tile([128, 128], I32, tag="xint")
                nc.vector.tensor_copy(xint[:], xsp8[:])
                x0f = ixp.tile([128, 128], F32, tag="x0f")
                nc.vector.tensor_copy(x0f[:], xint[:])
                if CAST_RNE:
                    nc.vector.tensor_scalar(xsp8[:], xsp8[:], 0.5, None, ALU.add)
                fx = ixp.tile([128, 128], F32, tag="fx")
                nc.vector.tensor_tensor(fx[:], xsp8[:], x0f[:], ALU.subtract)
                xi = ixp.tile([128, 128], F32, tag="xi")
                nc.vector.tensor_scalar(xi[:], x0f[:], 6.0, 138.0, ALU.max, ALU.min)

                # flat = (yi-6)*LC + (xi-6) = yi*LC + xi - 6*(LC+1)
                fl = ixp.tile([128, 128], F32, tag="fl")
                nc.vector.tensor_scalar(fl[:], yi[:], float(LC), float(6 * (LC + 1)),
                                        ALU.mult, ALU.subtract)
                nc.vector.tensor_tensor(fl[:], fl[:], xi[:], ALU.add)
                # wrap: OUT_pp[P, s] = fl[pp*16 + P%16, s] via selection matmuls,
                # then strided-cast into the wrapped idx tile (8x replicated)
                for pp in range(8):
                    wps = wrp_pool.tile([128, 128], F32, tag="wrps")
                    nc.tensor.matmul(wps[:], msel_v[:, pp, :], fl[:],
                                     start=True, stop=True)
                    dstw = bass.AP(W16all.tensor,
                                   W16all[:].offset + k * (SG * 64) + pp,
                                   [[KK * SG * 64, 128], [64, SG], [8, 8]])
                    nc.vector.tensor_copy(dstw, wps[:].rearrange(
                        "p (g u) -> p g u", g=SG))

                wy0 = ixp.tile([128, 128], F32, tag="wy0")
                nc.vector.tensor_scalar(wy0[:], fy[:], -1.0, 1.0, ALU.mult, ALU.add)
                wx0 = ixp.tile([128, 128], F32, tag="wx0")
                nc.vector.tensor_scalar(wx0[:], fx[:], -1.0, 1.0, ALU.mult, ALU.add)
                # patch quadrant order: [TL(y0,x0), BL(y1,x0), TR(y0,x1), BR]
                gs = lambda t: t[:].rearrange("p (g s) -> p g s", g=SG)
                nc.vector.tensor_tensor(W4v[:, k, :, 0, :], gs(wy0), gs(wx0), ALU.mult)
                nc.vector.tensor_tensor(W4v[:, k, :, 1, :], gs(fy), gs(wx0), ALU.mult)
                nc.vector.tensor_tensor(W4v[:, k, :, 2, :], gs(wy0), gs(fx), ALU.mult)
                nc.vector.tensor_tensor(W4v[:, k, :, 3, :], gs(fy), gs(fx), ALU.mult)

        # ---------------- phase C+D: idx bounce, gather, blend, conv ------
        with tc.tile_pool(name="wexps", bufs=2, space="PSUM") as wexp_pool, \
             tc.tile_pool(name="wtps", bufs=2, space="PSUM") as wtp_pool, \
             tc.tile_pool(name="trps2", bufs=2, space="PSUM") as trp2_pool, \
             tc.tile_pool(name="outps", bufs=2, space="PSUM") as outp_pool:

            ltab = ins["ltab"]
            lt_src = bass.AP(ltab.tensor, 0, [[128, LROWS - 2], [1, 256]])
            ni_reg = nc.gpsimd.to_reg(NI)
            wdt_v = wdt_sb[:].rearrange("p (i m) -> p i m", i=5)

            for g in range(SG):
                b2_tiles = []
                for (ka, kb) in KPAIRS:
                    b2 = b2_pool.tile([128, SP * 128], F32, tag="b2")
                    b2v = b2[:].rearrange("p (s c) -> p s c", c=128)
                    if kb is None:
                        nc.vector.memset(b2v[:, :, 64:128], 0.0)
                    for half, k in enumerate((ka, kb)):
                        if k is None:
                            continue
                        gout = gout_pool.tile([128, SP * 256], L_DT, tag="gout")
                        g3 = gout[:].rearrange("p (s e) -> p s e", e=256)
                        nc.gpsimd.dma_gather(
                            g3, lt_src, W16v[:, k, g, :], NI, ni_reg, 256, elem_step=128)
                        touch = touch_pool.tile([128, 2], L_DT, tag="touch")
                        touch_i = nc.vector.tensor_copy(touch[:], g3[:, 0, 0:2])

                        dst = b2v[:, :, half * 64:(half + 1) * 64]
                        tmp = btmp_pool.tile([128, SP * 64], F32, tag="btmp")
                        tmp3 = tmp[:].rearrange("p (s c) -> p s c", c=64)
                        for q in range(4):
                            wtp = wtp_pool.tile([SP, 128], F32, tag="wtp")
                            nc.tensor.transpose(
                                wtp[:], W4v[:, k, g, q, :], ident[:])
                            wts = wts_pool.tile([SP, 128], F32R, tag="wts")
                            nc.scalar.copy(wts[:], wtp[:])
                            we = wexp_pool.tile([128, SP * 64], F32, tag="wexp")
                            nc.tensor.matmul(
                                we[:], wts[:], id8ones[:], start=True, stop=True)
                            wv = we[:].rearrange("p (s c) -> p s c", c=64)
                            if q == 0:
                                bl_i = nc.vector.tensor_tensor(
                                    dst, g3[:, :, 0:64], wv, ALU.mult)
                                add_dep_helper(bl_i.ins, touch_i.ins, sync=False,
                                               reason="order blend after gather-touch")
                            else:
                                nc.vector.tensor_tensor(
                                    tmp3, g3[:, :, q * 64:(q + 1) * 64], wv, ALU.mult)
                                nc.vector.tensor_tensor(dst, dst, tmp3, ALU.add)
                    b2_tiles.append(b2)

                osb = osb_pool.tile([64, SP * 128], F32, tag="osb")
                for jp in range(SP // 2):
                    op = outp_pool.tile([64, 256], F32, tag="outps")
                    for i, b2 in enumerate(b2_tiles):
                        b2v = b2[:].rearrange("p (s c) -> p s c", c=128)
                        tp = trp2_pool.tile([128, 256], F32, tag="tp2")
                        nc.tensor.transpose(tp[:, 0:128], b2v[:, 2 * jp, :],
                                            ident[:])
                        nc.tensor.transpose(tp[:, 128:256], b2v[:, 2 * jp + 1, :],
                                            ident[:])
                        rhs = rhs_pool.tile([128, 256], F32R, tag="rhs")
                        nc.scalar.copy(rhs[:], tp[:])
                        nc.tensor.matmul(op[:], wdt_v[:, i, :], rhs[:],
                                         start=(i == 0), stop=(i == 4))
                    nc.vector.tensor_copy(osb[:, jp * 256:(jp + 1) * 256], op[:])
                nc.sync.dma_start(outs["out"][:, g * NI:(g + 1) * NI], osb[:])


# ======================= host-side wrapper =======================

def prep_core_inputs(xb, w_offset, w_deform):
    """Build per-core device input arrays from one sample."""
    C_, Hh, Ww = xb.shape
    xp = np.zeros((C_, Hh + 2, Ww + 2), np.float32)
    xp[:, 1:-1, 1:-1] = xb
    xpad = np.ascontiguousarray(xp.reshape(C_, -1), dtype=np.float32)

    L = np.zeros((LR, LC, 2, C_), np.float32)
    for j in range(2):
        ys = np.arange(LR) - 2 + j
        ok = (ys >= 0) & (ys < Hh)
        L[ok, 2:2 + Ww, j, :] = xb[:, ys[ok], :].transpose(1, 2, 0)
    Lrows = np.zeros((LROWS, 2 * C_), np.float32)
    Lrows[:LR * LC] = L.reshape(LR * LC, 2 * C_)

    woff = np.ascontiguousarray(
        w_offset.reshape(18, C_, KK).transpose(2, 1, 0), dtype=np.float32)

    wd = w_deform.reshape(64, C_, KK)
    wdt = np.zeros((5, 128, 64), np.float32)
    for i, (ka, kb) in enumerate(KPAIRS):
        wdt[i, 0:64, :] = wd[:, :, ka].T
        if kb is not None:
            wdt[i, 64:128, :] = wd[:, :, kb].T

    id8 = np.kron(np.eye(SP, dtype=np.float32), np.ones((1, 64), np.float32))
    msel = np.zeros((8, 128, 128), np.float32)
    for pp in range(8):
        for P in range(128):
            msel[pp, pp * 16 + P % 16, P] = 1.0
    return {
        "xpad": xpad,
        "ltab": Lrows,
        "woff": woff,
        "wdt": wdt,
        "id8ones": np.ascontiguousarray(id8),
        "msel": msel,
    }


_NC_CACHE = {}


def _build_nc():
    if "nc" in _NC_CACHE:
        return _NC_CACHE["nc"]
    nc = bacc.Bacc("TRN2", target_bir_lowering=False, debug=False,
                   num_devices=NCORES)
    ins = {
        "xpad": nc.dram_tensor("xpad", [C, 130 * 130], F32, kind="ExternalInput").ap(),
        "ltab": nc.dram_tensor("ltab", [LROWS, 2 * C], L_DT, kind="ExternalInput").ap(),
        "woff": nc.dram_tensor("woff", [KK, C, 18], F32, kind="ExternalInput").ap(),
        "wdt": nc.dram_tensor("wdt", [5, 128, 64], F32, kind="ExternalInput").ap(),
        "id8ones": nc.dram_tensor("id8ones", [SP, SP * 64], F32, kind="ExternalInput").ap(),
        "msel": nc.dram_tensor("msel", [8, 128, 128], F32, kind="ExternalInput").ap(),
    }
    outs = {"out": nc.dram_tensor("out", [C, HW], F32, kind="ExternalOutput").ap()}
    with tile.TileContext(nc, trace_sim=False) as tc:
        build_kernel(tc, outs, ins)
    nc.compile()
    _NC_CACHE["nc"] = nc
    return nc


def kernel(x, w_offset, w_deform):
    x = np.asarray(x, dtype=np.float32)
    w_offset = np.asarray(w_offset, dtype=np.float32)
    w_deform = np.asarray(w_deform, dtype=np.float32)
    nc = _build_nc()
    in_maps = [prep_core_inputs(x[b], w_offset, w_deform) for b in range(B)]
    res = bass_utils.run_bass_kernel_spmd(nc, in_maps, core_ids=list(range(NCORES)))
    out = np.stack([res.results[b]["out"].reshape(C, H, W) for b in range(B)])
    return out.astype(np.float32)
